# revision 12
# baseline (speedup 1.0000x reference)
"""MoE (top-2, 8 experts) Trainium2 kernel — expert parallelism across 8 NeuronCores.

Strategy:
  * Host (numpy, fp32, matching the reference's routing math): gate logits ->
    softmax -> top-2 -> group token/expert pairs by expert ("all-to-all
    dispatch" realized as the host-side shard step of the full-IO contract).
  * Core e gets expert e's tokens, transposed to [D, C] (C = max token count
    over experts, identical on all cores -> one SPMD program), plus expert
    e's weights, all cast to bf16 (single pass; fp32 PSUM accumulate keeps
    absmax rel err ~3e-3, well under the 2e-2 gate).
    L1: hT[H, C] = relu(W1^T xT + b1) -> bf16 (one DVE op per m-block).
    L2: yT[D, C] = W2^T hT (bias b2 and gate probs applied on host).
  * Host: out[tok] += (yT[:, :c_e]^T + b2[e]) * p_e  ("all-to-all combine").

Single-pass bf16 runs the PE at 1 cycle/row: 256*C cycles @ 2.4 GHz ~ 116 us
for C~1086, vs 349 us for the 3-pass hi/lo scheme this replaces.

Self-contained: hardcodes shapes from the problem spec (B=4, S=1024, D=1024,
H=2048, E=8, top-2), computes routing capacity from the actual inputs.
"""

import contextlib
import ctypes
import sys
import types

import numpy as np
import ml_dtypes

B, S, D, H, E, TOPK = 4, 1024, 1024, 2048, 8, 2
N_CORES = 8
P = 128
KO1 = D // P   # 8  K-tiles in layer 1
M1 = H // P    # 16 M-tiles in layer 1
KO2 = H // P   # 16 K-tiles in layer 2
M2 = D // P    # 8  M-tiles in layer 2
NTMAX = 512    # PSUM fp32 bank is 512 fp32

BF16 = ml_dtypes.bfloat16


def _install_axon_ntff_hook():
    """This image's antenv lacks axon_hooks; inject the ctypes NTFF profiling
    hook so run_bass_kernel_spmd(trace=True) works instead of crashing."""
    try:
        import antenv.axon_hooks  # noqa: F401
        return
    except ImportError:
        pass
    try:
        import antenv
    except ImportError:
        return

    so_path = "/opt/axon/libaxon_pjrt.so"
    try:
        lib = ctypes.CDLL(so_path)
    except OSError:
        lib = None
    hook = None
    if lib is not None and hasattr(lib, "axon_start_nrt_profile"):
        lib.axon_start_nrt_profile.argtypes = [
            ctypes.POINTER(ctypes.c_int64),
            ctypes.c_size_t,
        ]
        lib.axon_start_nrt_profile.restype = ctypes.c_int64
        lib.axon_stop_nrt_profile.argtypes = [ctypes.c_char_p]
        lib.axon_stop_nrt_profile.restype = ctypes.c_int64

        @contextlib.contextmanager
        def hook(output_dir, device_ids):
            import jax

            jax.devices()
            if device_ids:
                ids = (ctypes.c_int64 * len(device_ids))(*device_ids)
                rc = lib.axon_start_nrt_profile(ids, len(device_ids))
            else:
                rc = lib.axon_start_nrt_profile(None, 0)
            if rc != 0:
                raise RuntimeError(f"axon_start_nrt_profile rc={rc}")
            try:
                yield
            finally:
                n = lib.axon_stop_nrt_profile(str(output_dir).encode())
                print(f"profile: {n} file(s) -> {output_dir}", file=sys.stderr)

    mod = types.ModuleType("antenv.axon_hooks")
    state = {"hook": hook}
    mod.set_axon_ntff_profile_hook = lambda h: state.__setitem__("hook", h)
    mod.get_axon_ntff_profile_hook = lambda: state["hook"]
    sys.modules["antenv.axon_hooks"] = mod
    antenv.axon_hooks = mod


def _patch_upload_artifacts():
    """Trace post-processing uploads artifacts to S3; make failures non-fatal."""
    from concourse import bass_utils

    orig = bass_utils.upload_artifacts
    if getattr(orig, "_moe_safe", False):
        return

    def safe_upload(tmpdir):
        try:
            return orig(tmpdir)
        except Exception:
            return f"file://{tmpdir}"

    safe_upload._moe_safe = True
    bass_utils.upload_artifacts = safe_upload


def _chunks(C):
    """Even-width chunks (max NTMAX): keeps every chunk wide enough that
    LDWEIGHTS (~107ns) hides under the matmul stream (w/2.4 ns)."""
    nch = -(-C // NTMAX)
    base, rem = divmod(C, nch)
    out = []
    c0 = 0
    for i in range(nch):
        w = base + (1 if i < rem else 0)
        out.append((c0, w))
        c0 += w
    return out


_PROGRAM_CACHE = {}


def _build_program(C):
    """Per-core bass program: 2-layer FFN on [D, C] tokens, single-pass bf16.

    Host pre-tiles weights/activations so every DMA is one contiguous run per
    partition. SBUF weight layout [P, M, KO, P] mirrors the host tiling, so a
    per-m-block DMA is a straight [P, KO*P] copy and the matmul lhsT slice
    [:, m, ko, :] is contiguous. Weight m-blocks alternate between the Sync
    and Scalar HWDGE queues; x chunks split across both. A few matmuls on a
    zeroed dummy tile run while DMAs land, so the PE HAM clock-gate is already
    released (2.4 GHz) when the real stream starts.
    """
    import concourse.tile as tile
    from concourse import bacc, mybir

    chunks = _chunks(C)
    nch = len(chunks)

    nc = bacc.Bacc(None, debug=False)
    bf = mybir.dt.bfloat16
    f32 = mybir.dt.float32
    Alu = mybir.AluOpType

    x_d = nc.dram_tensor("x", [P * KO1 * C], bf, kind="ExternalInput")
    w1_d = nc.dram_tensor("w1", [M1, P, KO1 * P], bf, kind="ExternalInput")
    w2_d = nc.dram_tensor("w2", [M2, P, KO2 * P], bf, kind="ExternalInput")
    b1_d = nc.dram_tensor("b1r", [P, M1], f32, kind="ExternalInput")
    # Output as bf16: halves store traffic and shortens the final-store
    # drain; host combine upcasts. Adds ~3e-4 to the absmax rel err.
    yT_d = nc.dram_tensor("yT", [D, C], bf, kind="ExternalOutput")

    with tile.TileContext(nc) as tc:
        with (
            tc.tile_pool(name="wpool", bufs=2) as wpool,
            tc.tile_pool(name="xpool", bufs=3) as xpool,
            tc.tile_pool(name="hpool", bufs=2) as hpool,
            tc.tile_pool(name="ypool", bufs=4) as ypool,
            tc.tile_pool(name="bpool", bufs=1) as bpool,
            tc.tile_pool(name="pspool", bufs=6, space="PSUM") as pspool,
            tc.tile_pool(name="wuppool", bufs=1, space="PSUM") as wuppool,
        ):
            # PE warmup: matmuls on a zeroed tile, no DMA deps, result unread.
            # ~34 cold N=128 matmuls ~ 3.6us: spans the HAM SHORT window so the
            # real stream starts at 2.4 GHz, timed to end as the critical DMAs
            # (x chunk 0 + first W1 m-blocks, ~1.25MB dual-queue) land.
            # 64 MMs: ~32 run cold (3.4us, spans the HAM SHORT window and
            # releases the clock gate), the rest warm (~1.8us) — sized so the
            # warmup ends just as the last ramp-critical DMA lands. The real
            # stream then runs gap-free from its first matmul: any multi-us
            # stall after stream start risks a mostly-idle HAM window, which
            # re-throttles the PE to 1.2 GHz for 3.4us+ (observed).
            wup_src = bpool.tile([P, P], bf, tag="wup_src")
            nc.vector.memset(wup_src, 0.0)
            wup_ps = wuppool.tile([P, P], f32, tag="wup_ps")
            for i in range(64):
                nc.tensor.matmul(
                    wup_ps, wup_src, wup_src,
                    start=(i == 0), stop=(i == 63),
                )

            x_tiles = {}

            def load_x(ci):
                c0, w = chunks[ci]
                off = P * KO1 * c0
                x_t = xpool.tile([P, KO1, w], bf, tag="x")
                src = x_d[off : off + P * KO1 * w].rearrange("(p k) -> p k", p=P)
                # Split across both HWDGE queues; first half (ko 0-3) lands
                # first and unblocks the opening matmuls of the chunk.
                hw = (KO1 // 2) * w
                nc.sync.dma_start(out=x_t[:, : KO1 // 2], in_=src[:, :hw])
                nc.scalar.dma_start(out=x_t[:, KO1 // 2 :], in_=src[:, hw:])
                x_tiles[ci] = x_t

            # Both layers' weights stay resident (64KB/partition), per-m-block
            # single-run DMAs, alternating Sync/Scalar queues.
            w1 = wpool.tile([P, M1, KO1, P], bf, tag="w")
            w2 = wpool.tile([P, M2, KO2, P], bf, tag="w")

            # Ramp-critical ordering (queues deliver ~165-185GB/s each, data
            # starting ~1.5us after the first trigger): the stream's opening
            # m-block needs x chunk 0 (740KB) + w1[0]; m-block k then needs
            # w1[k] every ~1.2us. Interleave both queues in consumption
            # order so no critical piece sits behind bulk.
            c0w = chunks[0][1]
            x0_t = xpool.tile([P, KO1, c0w], bf, tag="x")
            src0 = x_d[: P * KO1 * c0w].rearrange("(p k) -> p k", p=P)
            # Measured: the Sync ring sustains ~205GB/s from ~8.8us, the
            # Scalar ring ~170GB/s from ~9.9us, completion sem ~0.6-1.3us
            # after last byte. Assign pieces so each lands before its
            # consumption deadline (stream starts ~12.5us, m-block k of L1
            # consumed at ~12.5 + 1.24k us).
            nc.scalar.dma_start(out=w1[:, 0], in_=w1_d[0])
            nc.sync.dma_start(out=x0_t[:, :3], in_=src0[:, : 3 * c0w])
            nc.scalar.dma_start(out=x0_t[:, 6:], in_=src0[:, 6 * c0w :])
            nc.sync.dma_start(out=x0_t[:, 3:6], in_=src0[:, 3 * c0w : 6 * c0w])
            nc.sync.dma_start(out=w1[:, 1], in_=w1_d[1])
            nc.scalar.dma_start(out=w1[:, 2], in_=w1_d[2])
            x_tiles[0] = x0_t

            # b1 rides the GPSIMD (SWDGE) queue: keeps the two HWDGE rings
            # clear for the ramp-critical pieces. Needed only by the first
            # DVE op (~1.5us after the stream starts) — SWDGE latency is fine.
            b1_sb = bpool.tile([P, M1], f32, tag="b1")
            nc.gpsimd.dma_start(out=b1_sb, in_=b1_d[:, :])

            # Remaining weights: odd m on the (slower) Scalar ring has ample
            # slack; Sync tops up the evens.
            nc.scalar.dma_start(out=w1[:, 3], in_=w1_d[3])
            for m in range(4, M1):
                q = nc.sync if m % 2 == 0 else nc.scalar
                q.dma_start(out=w1[:, m], in_=w1_d[m])
            for m in range(M2):
                q = nc.sync if m % 2 == 0 else nc.scalar
                q.dma_start(out=w2[:, m], in_=w2_d[m])

            # Fused per-chunk L1+L2: no cross-phase PE gap, weights never swap.
            for ci in range(nch):
                if ci + 1 < nch:
                    load_x(ci + 1)
                x_t = x_tiles.pop(ci)
                c0, w = chunks[ci]
                csl = slice(c0, c0 + w)
                hT = hpool.tile([P, KO2, w], bf, tag="h")

                # ---- Layer 1: hT = bf16(relu(W1^T @ xT + b1)) ----
                for m in range(M1):
                    ps = pspool.tile([P, w], f32, tag="ps")
                    for ko in range(KO1):
                        nc.tensor.matmul(
                            ps, w1[:, m, ko], x_t[:, ko, :],
                            start=(ko == 0), stop=(ko == KO1 - 1),
                        )
                    # hT = bf16(max(ps + b1, 0)) in one DVE op straight from PSUM
                    nc.vector.tensor_scalar(
                        out=hT[:, m, :], in0=ps,
                        scalar1=b1_sb[:, m : m + 1], scalar2=0.0,
                        op0=Alu.add, op1=Alu.max,
                    )

                # ---- Layer 2: yT = W2^T @ hT ----
                for m in range(M2):
                    msl = slice(m * P, (m + 1) * P)
                    ps = pspool.tile([P, w], f32, tag="ps")
                    for ko in range(KO2):
                        nc.tensor.matmul(
                            ps, w2[:, m, ko], hT[:, ko, :],
                            start=(ko == 0), stop=(ko == KO2 - 1),
                        )
                    yt = ypool.tile([P, w], bf, tag="y")
                    nc.vector.tensor_copy(out=yt, in_=ps)
                    q = nc.sync if m % 2 == 0 else nc.scalar
                    q.dma_start(out=yT_d[msl, csl], in_=yt)

    nc.finalize()
    return nc


LAST_EXEC_NS = None
LAST_TRACE = None


def kernel(x, Wg, W1, b1, W2, b2):
    import os

    global LAST_EXEC_NS, LAST_TRACE

    _install_axon_ntff_hook()
    _patch_upload_artifacts()
    from concourse.bass_utils import run_bass_kernel_spmd

    x = np.asarray(x, np.float32)
    Wg = np.asarray(Wg, np.float32)
    W1 = np.asarray(W1, np.float32)
    b1 = np.asarray(b1, np.float32)
    W2 = np.asarray(W2, np.float32)
    b2 = np.asarray(b2, np.float32)

    N = B * S
    xm = np.ascontiguousarray(x.reshape(N, D))

    # --- host routing: identical math to the reference (fp32) ---
    logits = xm @ Wg
    mx = logits.max(-1, keepdims=True)
    ex = np.exp(logits - mx)
    probs = ex / ex.sum(-1, keepdims=True)
    idx = np.argsort(-probs, axis=-1, kind="stable")[:, :TOPK]  # top-2, desc
    p2 = np.take_along_axis(probs, idx, axis=-1)

    toks_per_e = []
    probs_per_e = []
    for e in range(E):
        toks, slots = np.where(idx == e)
        toks_per_e.append(toks)
        probs_per_e.append(p2[toks, slots])
    counts = np.array([len(t) for t in toks_per_e])
    C = int(max(counts.max(), 1))  # exact capacity: no padded columns
    chunks = _chunks(C)

    def _tile_w1(w):  # [D, H] -> [M1, P, KO1*P]
        return np.ascontiguousarray(
            w.reshape(KO1, P, M1, P).transpose(2, 1, 0, 3).reshape(M1, P, KO1 * P))

    def _tile_w2(w):  # [H, D] -> [M2, P, KO2*P]
        return np.ascontiguousarray(
            w.reshape(KO2, P, M2, P).transpose(2, 1, 0, 3).reshape(M2, P, KO2 * P))

    def _tile_x(xsT):  # [D, C] -> flat [P*KO1*C], chunk-major [P, KO1, w] blocks
        parts = []
        for c0, w in chunks:
            blk = xsT[:, c0:c0 + w].reshape(KO1, P, w).transpose(1, 0, 2)
            parts.append(np.ascontiguousarray(blk).reshape(-1))
        return np.concatenate(parts)

    # --- per-core inputs ---
    xmT = np.ascontiguousarray(xm.T)  # [D, N]
    in_maps = []
    for e in range(E):
        toks = toks_per_e[e]
        xsT = np.zeros((D, C), np.float32)
        xsT[:, : len(toks)] = xmT[:, toks]
        b1r = np.ascontiguousarray(b1[e].reshape(M1, P).T)  # [128, 16]
        in_maps.append({
            "x": _tile_x(xsT.astype(BF16)),
            "w1": _tile_w1(W1[e].astype(BF16)),
            "w2": _tile_w2(W2[e].astype(BF16)),
            "b1r": b1r,
        })

    if C not in _PROGRAM_CACHE:
        _PROGRAM_CACHE[C] = _build_program(C)
    nc = _PROGRAM_CACHE[C]

    trace = os.environ.get("BASS_MOE_TRACE", "").strip() in ("1", "true", "yes")
    kw = {}
    if trace:
        kw["trace"] = True
        tdir = os.environ.get("BASS_MOE_TRACE_DIR")
        if tdir:
            kw["tmpdir"] = tdir
    res = run_bass_kernel_spmd(nc, in_maps, core_ids=list(range(N_CORES)), **kw)
    LAST_EXEC_NS = res.exec_time_ns
    LAST_TRACE = res.instructions_and_trace[1] if res.instructions_and_trace else None

    # --- host combine: bias2 + gates + scatter-add ---
    out = np.zeros((N, D), np.float32)
    for e in range(E):
        toks = toks_per_e[e]
        if len(toks) == 0:
            continue
        y = res.results[e]["yT"][:, : len(toks)].T.astype(np.float32)  # [c_e, D]
        out[toks] += (y + b2[e]) * probs_per_e[e][:, None]
    return out.reshape(B, S, D)


# revision 15
# speedup vs baseline: 1.0018x; 1.0018x over previous
"""MoE (top-2, 8 experts) Trainium2 kernel — expert parallelism across 8 NeuronCores.

Strategy:
  * Host (numpy, fp32, matching the reference's routing math): gate logits ->
    softmax -> top-2 -> group token/expert pairs by expert ("all-to-all
    dispatch" realized as the host-side shard step of the full-IO contract).
  * Core e gets expert e's tokens, transposed to [D, C] (C = max token count
    over experts, identical on all cores -> one SPMD program), plus expert
    e's weights, all cast to bf16 (single pass; fp32 PSUM accumulate keeps
    absmax rel err ~3e-3, well under the 2e-2 gate).
    L1: hT[H, C] = relu(W1^T xT + b1) -> bf16 (one DVE op per m-block).
    L2: yT[D, C] = W2^T hT, stored bf16 (bias b2 + gate probs on host).
  * Host: out[tok] += (yT[:, :c_e]^T + b2[e]) * p_e  ("all-to-all combine").

Single-pass bf16 runs the PE at 1 cycle/row: 256*C cycles @ 2.4 GHz ~ 116 us
for C~1086, vs 349 us for the 3-pass hi/lo scheme this replaces. Measured
~137-139 us end-to-end: ~7 us fixed NEFF prologue, ~5.5 us DMA ramp (hidden
behind PE warmup that also releases the HAM clock-gate), gap-free matmul
stream at ~155 ns/MM, ~5 us store-drain + fixed epilogue. Absmax rel err
3.3e-3 (gate: 2e-2).

Self-contained: hardcodes shapes from the problem spec (B=4, S=1024, D=1024,
H=2048, E=8, top-2), computes routing capacity from the actual inputs.
"""

import contextlib
import ctypes
import sys
import types

import numpy as np
import ml_dtypes

B, S, D, H, E, TOPK = 4, 1024, 1024, 2048, 8, 2
N_CORES = 8
P = 128
KO1 = D // P   # 8  K-tiles in layer 1
M1 = H // P    # 16 M-tiles in layer 1
KO2 = H // P   # 16 K-tiles in layer 2
M2 = D // P    # 8  M-tiles in layer 2
NTMAX = 512    # PSUM fp32 bank is 512 fp32

BF16 = ml_dtypes.bfloat16


def _install_axon_ntff_hook():
    """This image's antenv lacks axon_hooks; inject the ctypes NTFF profiling
    hook so run_bass_kernel_spmd(trace=True) works instead of crashing."""
    try:
        import antenv.axon_hooks  # noqa: F401
        return
    except ImportError:
        pass
    try:
        import antenv
    except ImportError:
        return

    so_path = "/opt/axon/libaxon_pjrt.so"
    try:
        lib = ctypes.CDLL(so_path)
    except OSError:
        lib = None
    hook = None
    if lib is not None and hasattr(lib, "axon_start_nrt_profile"):
        lib.axon_start_nrt_profile.argtypes = [
            ctypes.POINTER(ctypes.c_int64),
            ctypes.c_size_t,
        ]
        lib.axon_start_nrt_profile.restype = ctypes.c_int64
        lib.axon_stop_nrt_profile.argtypes = [ctypes.c_char_p]
        lib.axon_stop_nrt_profile.restype = ctypes.c_int64

        @contextlib.contextmanager
        def hook(output_dir, device_ids):
            import jax

            jax.devices()
            if device_ids:
                ids = (ctypes.c_int64 * len(device_ids))(*device_ids)
                rc = lib.axon_start_nrt_profile(ids, len(device_ids))
            else:
                rc = lib.axon_start_nrt_profile(None, 0)
            if rc != 0:
                raise RuntimeError(f"axon_start_nrt_profile rc={rc}")
            try:
                yield
            finally:
                n = lib.axon_stop_nrt_profile(str(output_dir).encode())
                print(f"profile: {n} file(s) -> {output_dir}", file=sys.stderr)

    mod = types.ModuleType("antenv.axon_hooks")
    state = {"hook": hook}
    mod.set_axon_ntff_profile_hook = lambda h: state.__setitem__("hook", h)
    mod.get_axon_ntff_profile_hook = lambda: state["hook"]
    sys.modules["antenv.axon_hooks"] = mod
    antenv.axon_hooks = mod


def _patch_upload_artifacts():
    """Trace post-processing uploads artifacts to S3; make failures non-fatal."""
    from concourse import bass_utils

    orig = bass_utils.upload_artifacts
    if getattr(orig, "_moe_safe", False):
        return

    def safe_upload(tmpdir):
        try:
            return orig(tmpdir)
        except Exception:
            return f"file://{tmpdir}"

    safe_upload._moe_safe = True
    bass_utils.upload_artifacts = safe_upload


def _chunks(C):
    """Even-width chunks (max NTMAX): keeps every chunk wide enough that
    LDWEIGHTS (~107ns) hides under the matmul stream (w/2.4 ns)."""
    nch = -(-C // NTMAX)
    base, rem = divmod(C, nch)
    out = []
    c0 = 0
    for i in range(nch):
        w = base + (1 if i < rem else 0)
        out.append((c0, w))
        c0 += w
    return out


_PROGRAM_CACHE = {}


def _build_program(C):
    """Per-core bass program: 2-layer FFN on [D, C] tokens, single-pass bf16.

    Host pre-tiles weights/activations so every DMA is one contiguous run per
    partition. SBUF weight layout [P, M, KO, P] mirrors the host tiling, so a
    per-m-block DMA is a straight [P, KO*P] copy and the matmul lhsT slice
    [:, m, ko, :] is contiguous. Weight m-blocks alternate between the Sync
    and Scalar HWDGE queues; x chunks split across both. A few matmuls on a
    zeroed dummy tile run while DMAs land, so the PE HAM clock-gate is already
    released (2.4 GHz) when the real stream starts.
    """
    import concourse.tile as tile
    from concourse import bacc, mybir

    chunks = _chunks(C)
    nch = len(chunks)

    nc = bacc.Bacc(None, debug=False)
    bf = mybir.dt.bfloat16
    f32 = mybir.dt.float32
    Alu = mybir.AluOpType

    x_d = nc.dram_tensor("x", [P * KO1 * C], bf, kind="ExternalInput")
    w1_d = nc.dram_tensor("w1", [M1, P, KO1 * P], bf, kind="ExternalInput")
    w2_d = nc.dram_tensor("w2", [M2, P, KO2 * P], bf, kind="ExternalInput")
    b1_d = nc.dram_tensor("b1r", [P, M1], f32, kind="ExternalInput")
    # Output as bf16: halves store traffic and shortens the final-store
    # drain; host combine upcasts. Adds ~3e-4 to the absmax rel err.
    yT_d = nc.dram_tensor("yT", [D, C], bf, kind="ExternalOutput")

    with tile.TileContext(nc) as tc:
        with (
            tc.tile_pool(name="wpool", bufs=2) as wpool,
            tc.tile_pool(name="xpool", bufs=3) as xpool,
            tc.tile_pool(name="hpool", bufs=2) as hpool,
            tc.tile_pool(name="ypool", bufs=4) as ypool,
            tc.tile_pool(name="bpool", bufs=1) as bpool,
            tc.tile_pool(name="pspool", bufs=6, space="PSUM") as pspool,
            tc.tile_pool(name="wuppool", bufs=1, space="PSUM") as wuppool,
        ):
            # PE warmup: matmuls on a zeroed tile, no DMA deps, result unread.
            # 64 MMs: ~32 run cold (3.4us, spans the HAM SHORT window and
            # releases the clock gate), the rest warm (~1.8us) — sized so the
            # warmup ends just as the last ramp-critical DMA lands. The real
            # stream then runs gap-free from its first matmul: any multi-us
            # stall after stream start risks a mostly-idle HAM window, which
            # re-throttles the PE to 1.2 GHz for 3.4us+ (observed).
            wup_src = bpool.tile([P, P], bf, tag="wup_src")
            nc.vector.memset(wup_src, 0.0)
            wup_ps = wuppool.tile([P, P], f32, tag="wup_ps")
            for i in range(64):
                nc.tensor.matmul(
                    wup_ps, wup_src, wup_src,
                    start=(i == 0), stop=(i == 63),
                )

            x_tiles = {}

            def load_x(ci):
                c0, w = chunks[ci]
                off = P * KO1 * c0
                x_t = xpool.tile([P, KO1, w], bf, tag="x")
                src = x_d[off : off + P * KO1 * w].rearrange("(p k) -> p k", p=P)
                # Split across both HWDGE queues; first half (ko 0-3) lands
                # first and unblocks the opening matmuls of the chunk.
                hw = (KO1 // 2) * w
                nc.sync.dma_start(out=x_t[:, : KO1 // 2], in_=src[:, :hw])
                nc.scalar.dma_start(out=x_t[:, KO1 // 2 :], in_=src[:, hw:])
                x_tiles[ci] = x_t

            # Both layers' weights stay resident (64KB/partition), per-m-block
            # single-run DMAs, alternating Sync/Scalar queues.
            w1 = wpool.tile([P, M1, KO1, P], bf, tag="w")
            w2 = wpool.tile([P, M2, KO2, P], bf, tag="w")

            # Ramp-critical ordering. Measured: the Sync ring sustains
            # ~205GB/s from ~8.8us, the Scalar ring ~170GB/s from ~9.9us,
            # completion sem ~0.6-1.3us after the last byte. The stream's
            # opening m-block needs x chunk 0 (740KB) + w1[0]; m-block k of
            # L1 is then consumed at ~(stream start + 1.24k us). Assign
            # pieces so each lands before its deadline on either ring.
            c0w = chunks[0][1]
            x0_t = xpool.tile([P, KO1, c0w], bf, tag="x")
            src0 = x_d[: P * KO1 * c0w].rearrange("(p k) -> p k", p=P)
            nc.scalar.dma_start(out=w1[:, 0], in_=w1_d[0])
            nc.sync.dma_start(out=x0_t[:, :3], in_=src0[:, : 3 * c0w])
            nc.scalar.dma_start(out=x0_t[:, 6:], in_=src0[:, 6 * c0w :])
            nc.sync.dma_start(out=x0_t[:, 3:6], in_=src0[:, 3 * c0w : 6 * c0w])
            nc.sync.dma_start(out=w1[:, 1], in_=w1_d[1])
            nc.scalar.dma_start(out=w1[:, 2], in_=w1_d[2])
            x_tiles[0] = x0_t

            # b1 rides the GPSIMD (SWDGE) queue: keeps the two HWDGE rings
            # clear for the ramp-critical pieces. Needed only by the first
            # DVE op (~1.5us after the stream starts) — SWDGE latency is fine.
            b1_sb = bpool.tile([P, M1], f32, tag="b1")
            nc.gpsimd.dma_start(out=b1_sb, in_=b1_d[:, :])

            # Remaining weights: odd m on the (slower) Scalar ring has ample
            # slack; Sync tops up the evens.
            nc.scalar.dma_start(out=w1[:, 3], in_=w1_d[3])
            for m in range(4, M1):
                q = nc.sync if m % 2 == 0 else nc.scalar
                q.dma_start(out=w1[:, m], in_=w1_d[m])
            for m in range(M2):
                q = nc.sync if m % 2 == 0 else nc.scalar
                q.dma_start(out=w2[:, m], in_=w2_d[m])

            # Fused per-chunk L1+L2: no cross-phase PE gap, weights never swap.
            for ci in range(nch):
                if ci + 1 < nch:
                    load_x(ci + 1)
                x_t = x_tiles.pop(ci)
                c0, w = chunks[ci]
                csl = slice(c0, c0 + w)
                hT = hpool.tile([P, KO2, w], bf, tag="h")

                # ---- Layer 1: hT = bf16(relu(W1^T @ xT + b1)) ----
                for m in range(M1):
                    ps = pspool.tile([P, w], f32, tag="ps")
                    for ko in range(KO1):
                        nc.tensor.matmul(
                            ps, w1[:, m, ko], x_t[:, ko, :],
                            start=(ko == 0), stop=(ko == KO1 - 1),
                        )
                    # hT = bf16(max(ps + b1, 0)) in one DVE op straight from PSUM
                    nc.vector.tensor_scalar(
                        out=hT[:, m, :], in0=ps,
                        scalar1=b1_sb[:, m : m + 1], scalar2=0.0,
                        op0=Alu.add, op1=Alu.max,
                    )

                # ---- Layer 2: yT = W2^T @ hT ----
                for m in range(M2):
                    msl = slice(m * P, (m + 1) * P)
                    ps = pspool.tile([P, w], f32, tag="ps")
                    for ko in range(KO2):
                        nc.tensor.matmul(
                            ps, w2[:, m, ko], hT[:, ko, :],
                            start=(ko == 0), stop=(ko == KO2 - 1),
                        )
                    yt = ypool.tile([P, w], bf, tag="y")
                    nc.vector.tensor_copy(out=yt, in_=ps)
                    q = nc.sync if m % 2 == 0 else nc.scalar
                    q.dma_start(out=yT_d[msl, csl], in_=yt)

    nc.finalize()
    return nc


LAST_EXEC_NS = None
LAST_TRACE = None


def kernel(x, Wg, W1, b1, W2, b2):
    import os

    global LAST_EXEC_NS, LAST_TRACE

    _install_axon_ntff_hook()
    _patch_upload_artifacts()
    from concourse.bass_utils import run_bass_kernel_spmd

    x = np.asarray(x, np.float32)
    Wg = np.asarray(Wg, np.float32)
    W1 = np.asarray(W1, np.float32)
    b1 = np.asarray(b1, np.float32)
    W2 = np.asarray(W2, np.float32)
    b2 = np.asarray(b2, np.float32)

    N = B * S
    xm = np.ascontiguousarray(x.reshape(N, D))

    # --- host routing: identical math to the reference (fp32) ---
    logits = xm @ Wg
    mx = logits.max(-1, keepdims=True)
    ex = np.exp(logits - mx)
    probs = ex / ex.sum(-1, keepdims=True)
    idx = np.argsort(-probs, axis=-1, kind="stable")[:, :TOPK]  # top-2, desc
    p2 = np.take_along_axis(probs, idx, axis=-1)

    toks_per_e = []
    probs_per_e = []
    for e in range(E):
        toks, slots = np.where(idx == e)
        toks_per_e.append(toks)
        probs_per_e.append(p2[toks, slots])
    counts = np.array([len(t) for t in toks_per_e])
    C = int(max(counts.max(), 1))  # exact capacity: no padded columns
    chunks = _chunks(C)

    def _tile_w1(w):  # [D, H] -> [M1, P, KO1*P]
        return np.ascontiguousarray(
            w.reshape(KO1, P, M1, P).transpose(2, 1, 0, 3).reshape(M1, P, KO1 * P))

    def _tile_w2(w):  # [H, D] -> [M2, P, KO2*P]
        return np.ascontiguousarray(
            w.reshape(KO2, P, M2, P).transpose(2, 1, 0, 3).reshape(M2, P, KO2 * P))

    def _tile_x(xsT):  # [D, C] -> flat [P*KO1*C], chunk-major [P, KO1, w] blocks
        parts = []
        for c0, w in chunks:
            blk = xsT[:, c0:c0 + w].reshape(KO1, P, w).transpose(1, 0, 2)
            parts.append(np.ascontiguousarray(blk).reshape(-1))
        return np.concatenate(parts)

    # --- per-core inputs ---
    xmT = np.ascontiguousarray(xm.T)  # [D, N]
    in_maps = []
    for e in range(E):
        toks = toks_per_e[e]
        xsT = np.zeros((D, C), np.float32)
        xsT[:, : len(toks)] = xmT[:, toks]
        b1r = np.ascontiguousarray(b1[e].reshape(M1, P).T)  # [128, 16]
        in_maps.append({
            "x": _tile_x(xsT.astype(BF16)),
            "w1": _tile_w1(W1[e].astype(BF16)),
            "w2": _tile_w2(W2[e].astype(BF16)),
            "b1r": b1r,
        })

    if C not in _PROGRAM_CACHE:
        _PROGRAM_CACHE[C] = _build_program(C)
    nc = _PROGRAM_CACHE[C]

    trace = os.environ.get("BASS_MOE_TRACE", "").strip() in ("1", "true", "yes")
    kw = {}
    if trace:
        kw["trace"] = True
        tdir = os.environ.get("BASS_MOE_TRACE_DIR")
        if tdir:
            kw["tmpdir"] = tdir
    res = run_bass_kernel_spmd(nc, in_maps, core_ids=list(range(N_CORES)), **kw)
    LAST_EXEC_NS = res.exec_time_ns
    LAST_TRACE = res.instructions_and_trace[1] if res.instructions_and_trace else None

    # --- host combine: bias2 + gates + scatter-add ---
    out = np.zeros((N, D), np.float32)
    for e in range(E):
        toks = toks_per_e[e]
        if len(toks) == 0:
            continue
        y = res.results[e]["yT"][:, : len(toks)].T.astype(np.float32)  # [c_e, D]
        out[toks] += (y + b2[e]) * probs_per_e[e][:, None]
    return out.reshape(B, S, D)
